# revision 12
# baseline (speedup 1.0000x reference)
"""Trainium2 Bass kernel for a fused GRU cell.

Reference computation (per row b of a batch):
    z = sigmoid(x @ Wz + h @ Uz + bz)
    r = sigmoid(x @ Wr + h @ Ur + br)
    h_hat = tanh(x @ Wh + (r * h) @ Uh + bh)
    out = z * h + (1 - z) * h_hat

Shapes: x, h_prev [65536, 256] f32; six weights [256, 256]; three biases [256].

Strategy: data-parallel over the batch across 8 NeuronCores (8192 rows each).
On-core compute uses a transposed layout ([feature, row] on SBUF partitions)
so the tiny weight matrices are the stationary matmul operands and the
per-feature biases land on the partition axis (free via the ACT bias port).
The datapath is bf16 (cast during the DMA loads, widened back to f32 on the
store): bf16 matmuls get fast-weight-load and let walrus overlap LDWEIGHTS
with the matmul stream, and bf16 doubles DVE elementwise throughput. PSUM
accumulation stays fp32. Activations are transposed on the tensor engine
(128x128 blocks in PE transpose mode), gate math is split across
ACT/DVE/GPSIMD, and the result is transposed back before a contiguous store.
"""

import numpy as np

import concourse.mybir as mybir
import concourse.tile as tile
from concourse import bacc
from concourse.alu_op_type import AluOpType
from concourse.bass_utils import run_bass_kernel_spmd
from concourse.masks import make_identity

N_CORES = 8
B, D, U = 65536, 256, 256
ROWS_PER_CORE = B // N_CORES  # 8192
CHUNK = 512  # rows processed per pipeline iteration

F32 = mybir.dt.float32
BF16 = mybir.dt.bfloat16
AF = mybir.ActivationFunctionType

W_NAMES = ["Wz", "Uz", "Wr", "Ur", "Wh", "Uh"]
B_NAMES = ["bz", "br", "bh"]


def build_module(rows_per_core: int = ROWS_PER_CORE, iters: int = 1):
    """Emit + compile the per-core module. `iters` repeats the whole body
    (hardware loop) for wall-clock timing; the output is idempotent."""
    assert rows_per_core % CHUNK == 0
    nchunk = rows_per_core // CHUNK

    nc = bacc.Bacc("TRN2", target_bir_lowering=False, debug=False)

    x_d = nc.dram_tensor("x", [rows_per_core, D], F32, kind="ExternalInput").ap()
    h_d = nc.dram_tensor("h_prev", [rows_per_core, U], F32, kind="ExternalInput").ap()
    w_d = {n: nc.dram_tensor(n, [D, U], F32, kind="ExternalInput").ap() for n in W_NAMES}
    b_d = {n: nc.dram_tensor(n, [U], F32, kind="ExternalInput").ap() for n in B_NAMES}
    out_d = nc.dram_tensor("out", [rows_per_core, U], F32, kind="ExternalOutput").ap()

    with tile.TileContext(nc) as tc:
        with (
            tc.tile_pool(name="consts", bufs=1) as cpool,
            tc.tile_pool(name="loads", bufs=4) as lpool,
            tc.tile_pool(name="work", bufs=3) as wpool,
            tc.tile_pool(name="psum_tr", bufs=2, space="PSUM") as ptr,
            tc.tile_pool(name="psum_mm", bufs=6, space="PSUM") as pmm,
        ):
            # ---- constants (loaded once; weights cast f32->bf16 in the DMA) ----
            w_s = {}
            for n in W_NAMES:
                wt = cpool.tile([128, 2, U], BF16, name=f"ws_{n}")
                nc.gpsimd.dma_start(wt[:], w_d[n].rearrange("(kk p) n -> p kk n", p=128))
                w_s[n] = wt
            b_s = {}
            for n in B_NAMES:
                bt = cpool.tile([128, 2], F32, name=f"bs_{n}")
                nc.sync.dma_start(bt[:], b_d[n].rearrange("(u p) -> p u", p=128))
                b_s[n] = bt
            ident = cpool.tile([128, 128], BF16, name="ident")
            make_identity(nc, ident[:])

            def mm(out, w_name, kk, u, rhs, start, stop):
                lhsT = w_s[w_name][:, kk, u * 128:(u + 1) * 128]
                nc.tensor.matmul(out, lhsT, rhs, start=start, stop=stop)

            stash = {}

            def emit_front(c):
                nsub = CHUNK // 128  # 4
                # ---- load x/h chunk (HWDGE, f32), cast to bf16 on DVE/ACT ----
                x_nf = lpool.tile([128, nsub, D], F32, name="x_nf")
                h_nf = lpool.tile([128, nsub, U], F32, name="h_nf")
                xs = x_d[c * CHUNK:(c + 1) * CHUNK, :].rearrange("(s p) d -> p s d", p=128)
                hs = h_d[c * CHUNK:(c + 1) * CHUNK, :].rearrange("(s p) d -> p s d", p=128)
                nc.sync.dma_start(x_nf[:], xs)
                nc.sync.dma_start(h_nf[:], hs)
                x_n = lpool.tile([128, nsub, D], BF16, name="x_n")
                h_n = lpool.tile([128, nsub, U], BF16, name="h_n")
                nc.vector.tensor_copy(x_n[:], x_nf[:])
                nc.gpsimd.tensor_copy(h_n[:], h_nf[:])

                # ---- transpose x, h into [feat, row] layout (PE, bf16) ----
                xT = wpool.tile([128, 2, CHUNK], BF16, name="xT")
                hT = wpool.tile([128, 2, CHUNK], BF16, name="hT")
                for src_n, dstT, tag in ((x_n, xT, "ptx"), (h_n, hT, "pth")):
                    for kk in range(2):
                        ps_t = ptr.tile([128, CHUNK], BF16, name=f"ps_{tag}{kk}", tag="tr")
                        for s in range(nsub):
                            nc.tensor.transpose(
                                ps_t[:, s * 128:(s + 1) * 128],
                                src_n[:, s, kk * 128:(kk + 1) * 128],
                                ident[:],
                            )
                        if kk == 0 or tag == "pth":
                            nc.vector.tensor_copy(dstT[:, kk, :], ps_t[:])
                        else:
                            nc.scalar.copy(dstT[:, kk, :], ps_t[:])

                # ---- gate pre-activations: z, r (PSUM fp32) ----
                ps_z = [pmm.tile([128, CHUNK], F32, name="ps_z", tag="gates") for _ in range(2)]
                ps_r = [pmm.tile([128, CHUNK], F32, name="ps_r", tag="gates") for _ in range(2)]
                for u in range(2):
                    mm(ps_z[u][:], "Wz", 0, u, xT[:, 0, :], True, False)
                    mm(ps_z[u][:], "Wz", 1, u, xT[:, 1, :], False, False)
                    mm(ps_z[u][:], "Uz", 0, u, hT[:, 0, :], False, False)
                    mm(ps_z[u][:], "Uz", 1, u, hT[:, 1, :], False, True)
                for u in range(2):
                    mm(ps_r[u][:], "Wr", 0, u, xT[:, 0, :], True, False)
                    mm(ps_r[u][:], "Wr", 1, u, xT[:, 1, :], False, False)
                    mm(ps_r[u][:], "Ur", 0, u, hT[:, 0, :], False, False)
                    mm(ps_r[u][:], "Ur", 1, u, hT[:, 1, :], False, True)

                r_s = wpool.tile([128, 2, CHUNK], BF16, name="r_s")
                z_s = wpool.tile([128, 2, CHUNK], BF16, name="z_s")
                for u in range(2):
                    nc.scalar.activation(r_s[:, u, :], ps_r[u][:], AF.Sigmoid,
                                         bias=b_s["br"][:, u:u + 1])
                for u in range(2):
                    nc.scalar.activation(z_s[:, u, :], ps_z[u][:], AF.Sigmoid,
                                         bias=b_s["bz"][:, u:u + 1])

                # ---- rh = r * h (transposed layout) ----
                rh = wpool.tile([128, 2, CHUNK], BF16, name="rh")
                nc.vector.tensor_tensor(rh[:, 0, :], r_s[:, 0, :], hT[:, 0, :], AluOpType.mult)
                nc.gpsimd.tensor_tensor(rh[:, 1, :], r_s[:, 1, :], hT[:, 1, :], AluOpType.mult)
                stash[c] = (xT, hT, z_s, rh)

            def emit_back(c):
                nsub = CHUNK // 128  # 4
                xT, hT, z_s, rh = stash.pop(c)

                # ---- h_hat pre-activation: x @ Wh + rh @ Uh ----
                ps_g = [pmm.tile([128, CHUNK], F32, name="ps_g", tag="gates") for _ in range(2)]
                for u in range(2):
                    mm(ps_g[u][:], "Wh", 0, u, xT[:, 0, :], True, False)
                    mm(ps_g[u][:], "Wh", 1, u, xT[:, 1, :], False, False)
                    mm(ps_g[u][:], "Uh", 0, u, rh[:, 0, :], False, False)
                    mm(ps_g[u][:], "Uh", 1, u, rh[:, 1, :], False, True)

                hh = wpool.tile([128, 2, CHUNK], BF16, name="hh")
                for u in range(2):
                    nc.scalar.activation(hh[:, u, :], ps_g[u][:], AF.Tanh,
                                         bias=b_s["bh"][:, u:u + 1])

                # ---- blend: out = hh + z * (h - hh) ----
                dlt = wpool.tile([128, 2, CHUNK], BF16, name="dlt")
                hoT = wpool.tile([128, 2, CHUNK], BF16, name="hoT")
                nc.vector.tensor_tensor(dlt[:, 0, :], hT[:, 0, :], hh[:, 0, :], AluOpType.subtract)
                nc.vector.tensor_tensor(dlt[:, 1, :], hT[:, 1, :], hh[:, 1, :], AluOpType.subtract)
                nc.vector.tensor_tensor(dlt[:, 0, :], z_s[:, 0, :], dlt[:, 0, :], AluOpType.mult)
                nc.vector.tensor_tensor(dlt[:, 1, :], z_s[:, 1, :], dlt[:, 1, :], AluOpType.mult)
                nc.vector.tensor_tensor(hoT[:, 0, :], hh[:, 0, :], dlt[:, 0, :], AluOpType.add)
                nc.vector.tensor_tensor(hoT[:, 1, :], hh[:, 1, :], dlt[:, 1, :], AluOpType.add)

                # ---- transpose back to [row, feat] and store (widen to f32) ----
                ho_n = wpool.tile([128, nsub, U], BF16, name="ho_n")
                for pair in range(nsub // 2):
                    ps_o = pmm.tile([128, 2, U], BF16, name="ps_o", tag="gates")
                    for j in range(2):
                        s = pair * 2 + j
                        for u in range(2):
                            nc.tensor.transpose(
                                ps_o[:, j, u * 128:(u + 1) * 128],
                                hoT[:, u, s * 128:(s + 1) * 128],
                                ident[:],
                            )
                    nc.vector.tensor_copy(ho_n[:, 2 * pair:2 * pair + 2, :], ps_o[:])
                os_ = out_d[c * CHUNK:(c + 1) * CHUNK, :].rearrange("(s p) d -> p s d", p=128)
                nc.gpsimd.dma_start(os_, ho_n[:])

            def emit_all():
                emit_front(0)
                for c in range(nchunk):
                    if c + 1 < nchunk:
                        emit_front(c + 1)
                    emit_back(c)

            if iters == 1:
                emit_all()
            else:
                with tc.For_i(0, iters, 1):
                    emit_all()

    nc.compile()
    return nc


_NC_CACHE: dict = {}


def _get_module(rows_per_core: int = ROWS_PER_CORE, iters: int = 1):
    key = (rows_per_core, iters)
    if key not in _NC_CACHE:
        _NC_CACHE[key] = build_module(rows_per_core, iters)
    return _NC_CACHE[key]


def kernel(**inputs: np.ndarray) -> np.ndarray:
    x = np.ascontiguousarray(np.asarray(inputs["x"], dtype=np.float32))
    h = np.ascontiguousarray(np.asarray(inputs["h_prev"], dtype=np.float32))
    consts = {
        n: np.ascontiguousarray(np.asarray(inputs[n], dtype=np.float32))
        for n in W_NAMES + B_NAMES
    }
    assert x.shape == (B, D) and h.shape == (B, U)

    nc = _get_module()
    xs = x.reshape(N_CORES, ROWS_PER_CORE, D)
    hs = h.reshape(N_CORES, ROWS_PER_CORE, U)
    in_maps = [
        {"x": np.ascontiguousarray(xs[c]), "h_prev": np.ascontiguousarray(hs[c]), **consts}
        for c in range(N_CORES)
    ]
    res = run_bass_kernel_spmd(nc, in_maps, core_ids=list(range(N_CORES)))
    return np.concatenate([res.results[c]["out"] for c in range(N_CORES)], axis=0)
